# revision 12
# baseline (speedup 1.0000x reference)
"""Bi-directional RNN (scratch) Trainium2 kernel.

Strategy: time-chunk parallelism with burn-in. The tanh recurrence is
strongly contracting (|Jacobian| ~ 0.65), so a chunk started from h=0 a
burn-in of B steps early converges to the exact trajectory to fp32
precision. 8 cores = 2 directions x 4 time chunks of 1024 steps, fully
independent (no collectives).

Per-core program (SPMD, identical on all cores; direction handled by
host-side time reversal of the inputs):
  phase 1: xwT[h, t] = Wx @ x_chunk.T + bh          (fp32 GEMM)
  phase 2: h_t = tanh(xw_t + Wh h_{t-1})            (bf16 weight-stationary
           matvec chain, fp32 PSUM accumulate, xw injected into PSUM via an
           identity matmul)
  phase 3: y[t, o] = h_chunk @ Wy.T + by/2          (bf16 GEMM, fp32 out)

Host: slices/transposes inputs per core, runs the SPMD kernel via
run_bass_kernel_spmd, sums fwd+bwd partials.
"""
import sys

if '/opt/trn_rl_repo' not in sys.path:
    sys.path.insert(0, '/opt/trn_rl_repo')

import numpy as np
import ml_dtypes

import concourse.bass as bass
import concourse.mybir as mybir
import concourse.tile as tile
from concourse.bass import ds
from concourse.bass_utils import run_bass_kernel_spmd
from concourse.masks import make_identity
from bass_rust import ScopedClock, SemaphoreHandle

# ---------------------------------------------------------------------------
# Compat: this walrus cannot encode inline sync-waits on Drain/NoOp
# (NO_STRUCT codegen path).  Re-emit the Tile kernel-tail waits as
# standalone wait_ge instructions.
# ---------------------------------------------------------------------------


def _patched_drain_and_barrier(self, tick_clock, wait_clock):
    nop_inst = self.nc.sync.nop(nofuse=True, hint="tail_drain_waits")
    wait_clock.add_sem_waits(
        nop_inst.ins, ScopedClock({None: tick_clock.global_clock})
    )
    si = nop_inst.ins.sync_info
    waits = list(si.on_wait)
    si.on_wait = []
    for w in waits:
        self.nc.sync.wait_ge(SemaphoreHandle(w.ant_name, w.id), w.wait_value)
    self.nc.sync.drain()
    self.nc.all_engine_barrier()
    assert self.sems is not None
    popped = self.nc._tile_sem_poison_stack.pop()
    assert popped is self._sem_poison
    self.nc.clear_and_free_semaphores(list(self.sems.allocated().values()))
    self.nc.all_engine_barrier()


tile.TileContext._drain_and_barrier = _patched_drain_and_barrier

_ZERO_WAIT_OPS = (mybir.InstDrain, mybir.InstNoOp)


def _split_excess_waits(nc):
    """Hoist inline sync-waits beyond what this walrus can encode onto
    standalone InstEventSemaphore instructions placed just before the
    owning instruction (same engine, so semantics are identical)."""
    n_hoisted = 0
    for fn in nc.m.functions:
        for bb in fn.blocks:
            il = bb.instructions
            idx = 0
            while idx < len(il):
                inst = il[idx]
                si = inst.sync_info
                if si is None:
                    idx += 1
                    continue
                waits = list(si.on_wait)
                keep = 0 if isinstance(inst, _ZERO_WAIT_OPS) else 1
                if len(waits) <= keep:
                    idx += 1
                    continue
                hoist, remain = waits[keep:], waits[:keep]
                for k, wt in enumerate(hoist):
                    ev = mybir.InstEventSemaphore(
                        name=f"{inst.name}-hw{k}", ins=[], outs=[]
                    )
                    ev.engine = inst.engine
                    ev.sync_info = mybir.SyncInfo(on_wait=[wt], on_update=[])
                    il.insert(idx, ev)
                    idx += 1
                    n_hoisted += 1
                si.on_wait = remain
                idx += 1
    return n_hoisted

# ---------------------------------------------------------------------------
# Problem shapes (hardcoded per contest contract)
# ---------------------------------------------------------------------------
T, IN, H, OUT = 4096, 1024, 2048, 1024
N_CORES = 8
N_CHUNK = 4            # time chunks per direction
CH = T // N_CHUNK      # 1024 steps per chunk
BURN = 64              # burn-in steps (contracting recurrence)
S = CH + BURN          # steps executed per core
U = 16                 # recurrence steps per hardware-loop iteration

F32 = mybir.dt.float32
BF16 = mybir.dt.bfloat16

KB_IN = IN // 128      # 8   k-tiles over input dim
KB_H = H // 128        # 16  k-tiles over hidden dim
MB_H = H // 128        # 16  m-tiles over hidden dim


def _build_program(S=S, CH=CH, BURN=BURN, U=U):
    """One SPMD program: forward-RNN over an S-step chunk, burn-in dropped."""
    nc = bass.Bass()

    xT = nc.declare_dram_parameter("xT", [IN, S], F32, isOutput=False)
    WxT = nc.declare_dram_parameter("WxT", [IN, H], F32, isOutput=False)
    WhT = nc.declare_dram_parameter("WhT", [H, H], BF16, isOutput=False)
    WyT = nc.declare_dram_parameter("WyT", [H, OUT], BF16, isOutput=False)
    bh = nc.declare_dram_parameter("bh", [H], F32, isOutput=False)
    byh = nc.declare_dram_parameter("byh", [128, OUT], F32, isOutput=False)
    y = nc.declare_dram_parameter("y", [CH, OUT], F32, isOutput=True)

    with tile.TileContext(nc) as tc:
        with tc.tile_pool(name="persist", bufs=1) as persist:
            xwT_sb = persist.tile([128, KB_H, S], F32)      # xw, [h, t] layout
            h_sb = persist.tile([128, KB_H, S + 1], BF16)   # h history, [h, t]
            bh_sb = persist.tile([128, KB_H], F32)
            i_sb = persist.tile([128, 128], F32)            # identity (xw inject)
            byh_sb = persist.tile([128, OUT], F32)

            # static-address staging rings (dynamic-offset APs are limited
            # to a handful per engine per loop body by register pressure,
            # so the per-step tiles live at static addresses and one
            # dynamic copy per U-step block moves data in/out)
            h_stage = persist.tile([128, KB_H, U], BF16)
            xw_stage = persist.tile([128, KB_H, U], F32)

            nc.sync.dma_start(bh_sb[:, :], bh.rearrange("(kb p) -> p kb", p=128))
            nc.sync.dma_start(byh_sb[:, :], byh[:, :])
            make_identity(nc, i_sb[:, :])
            nc.gpsimd.memset(h_sb[:, :, 0:1], 0.0)
            nc.gpsimd.memset(h_stage[:, :, :], 0.0)

            # ---------------- phase 1: xwT = Wx @ x.T + bh ----------------
            with (
                tc.tile_pool(name="ph1", bufs=1) as ph1,
                tc.tile_pool(name="wx", bufs=4) as wxp,
                tc.tile_pool(name="ps1", bufs=2, space="PSUM") as ps1,
            ):
                xT_sb = ph1.tile([128, KB_IN, S], F32)
                for ib in range(KB_IN):
                    nc.sync.dma_start(
                        xT_sb[:, ib, :], xT[ib * 128:(ib + 1) * 128, :]
                    )
                t_chunks = []
                t0 = 0
                while t0 < S:
                    t_chunks.append((t0, min(512, S - t0)))
                    t0 += 512
                for hb in range(KB_H):
                    psl = [ps1.tile([128, n], F32, tag=f"ps{ci}", name=f"ps1_{hb}_{ci}")
                           for ci, (_, n) in enumerate(t_chunks)]
                    for ib in range(KB_IN):
                        wx_t = wxp.tile([128, 128], F32)
                        nc.sync.dma_start(
                            wx_t[:, :],
                            WxT[ib * 128:(ib + 1) * 128, hb * 128:(hb + 1) * 128],
                        )
                        for ci, (t0, n) in enumerate(t_chunks):
                            nc.tensor.matmul(
                                psl[ci][:, :],
                                wx_t[:, :],
                                xT_sb[:, ib, t0:t0 + n],
                                start=(ib == 0),
                                stop=(ib == KB_IN - 1),
                            )
                    for ci, (t0, n) in enumerate(t_chunks):
                        nc.vector.tensor_scalar_add(
                            xwT_sb[:, hb, t0:t0 + n],
                            psl[ci][:, :],
                            bh_sb[:, hb:hb + 1],
                        )

            # ---------------- phase 2: recurrence ----------------
            with (
                tc.tile_pool(name="wh", bufs=1) as whp,
                tc.tile_pool(name="ps2", bufs=2, space="PSUM") as ps2,
            ):
                wh_sb = whp.tile([128, KB_H, MB_H, 128], BF16)
                for kb in range(KB_H):
                    nc.sync.dma_start(
                        wh_sb[:, kb, :, :],
                        WhT[kb * 128:(kb + 1) * 128, :].rearrange(
                            "p (mb q) -> p mb q", q=128
                        ),
                    )

                n_blocks = S // U
                assert n_blocks * U == S
                with tc.For_i(
                    0, n_blocks, 1, hint_engines=(mybir.EngineType.PE,)
                ) as blk:
                    t0_sv = nc.snap(blk * U)
                    # prefetch this block's xw slice to a static address
                    nc.vector.tensor_copy(
                        xw_stage[:, :, :],
                        xwT_sb[:, :, ds(t0_sv, U)],
                    )
                    for i in range(U):
                        # rhs: h of previous step (last slot wraps to the
                        # previous block's final h; the back-edge barrier
                        # makes the cross-iteration reuse safe)
                        hin = (i - 1) % U
                        psum = ps2.tile([128, MB_H], F32)
                        # inject xw_t into PSUM (clears bank)
                        nc.tensor.matmul(
                            psum[:, :],
                            i_sb[:, :],
                            xw_stage[:, :, i:i + 1],
                            start=True,
                            stop=False,
                        )
                        for mb in range(MB_H):
                            for kb in range(KB_H):
                                nc.tensor.matmul(
                                    psum[:, mb:mb + 1],
                                    wh_sb[:, kb, mb, :],
                                    h_stage[:, kb, hin:hin + 1],
                                    start=False,
                                    stop=(mb == MB_H - 1 and kb == KB_H - 1),
                                )
                        nc.scalar.activation(
                            h_stage[:, :, i:i + 1],
                            psum[:, :],
                            mybir.ActivationFunctionType.Tanh,
                        )
                    # history copy for phase 3 (one dynamic AP per block)
                    nc.vector.tensor_copy(
                        h_sb[:, :, ds(t0_sv + 1, U)],
                        h_stage[:, :, :],
                    )

            # ---------------- phase 3: y = h.T @ WyT + by/2 ----------------
            with (
                tc.tile_pool(name="wy", bufs=1) as wyp,
                tc.tile_pool(name="yo", bufs=4) as yop,
                tc.tile_pool(name="ps3", bufs=4, space="PSUM") as ps3,
            ):
                wy_sb = wyp.tile([128, KB_H, OUT], BF16)
                for kb in range(KB_H):
                    nc.sync.dma_start(
                        wy_sb[:, kb, :], WyT[kb * 128:(kb + 1) * 128, :]
                    )
                for mt in range(CH // 128):
                    tbase = BURN + 1 + mt * 128
                    for oc in range(OUT // 512):
                        ps = ps3.tile([128, 512], F32)
                        for kb in range(KB_H):
                            nc.tensor.matmul(
                                ps[:, :],
                                h_sb[:, kb, tbase:tbase + 128],
                                wy_sb[:, kb, oc * 512:(oc + 1) * 512],
                                start=(kb == 0),
                                stop=(kb == KB_H - 1),
                            )
                        y_sb = yop.tile([128, 512], F32)
                        nc.vector.tensor_tensor(
                            y_sb[:, :],
                            ps[:, :],
                            byh_sb[:, oc * 512:(oc + 1) * 512],
                            mybir.AluOpType.add,
                        )
                        nc.sync.dma_start(
                            y[mt * 128:(mt + 1) * 128, oc * 512:(oc + 1) * 512],
                            y_sb[:, :],
                        )

    return nc


_PROGRAM_CACHE = {}


def _get_program():
    if "nc" not in _PROGRAM_CACHE:
        nc = _build_program()
        _split_excess_waits(nc)
        _PROGRAM_CACHE["nc"] = nc
    return _PROGRAM_CACHE["nc"]


def _make_in_maps(x, Wx_f, Wh_f, bh_f, Wx_b, Wh_b, bh_b, Wy_f, Wy_b, by):
    """Slice + transpose host-side into the 8 per-core input maps."""
    x = np.asarray(x, np.float32)
    byh = np.tile((np.asarray(by, np.float32) * 0.5)[None, :], (128, 1))
    byh = np.ascontiguousarray(byh)

    per_dir = {}
    for d, (Wx, Wh, bhv, Wy) in (
        ("f", (Wx_f, Wh_f, bh_f, Wy_f)),
        ("b", (Wx_b, Wh_b, bh_b, Wy_b)),
    ):
        per_dir[d] = {
            "WxT": np.ascontiguousarray(np.asarray(Wx, np.float32).T),
            "WhT": np.ascontiguousarray(
                np.asarray(Wh, np.float32).T.astype(ml_dtypes.bfloat16)
            ),
            "WyT": np.ascontiguousarray(
                np.asarray(Wy, np.float32).T.astype(ml_dtypes.bfloat16)
            ),
            "bh": np.ascontiguousarray(np.asarray(bhv, np.float32)),
        }

    x_rev = x[::-1]
    in_maps = []
    for c in range(N_CORES):
        d = "f" if c < N_CHUNK else "b"
        j = c % N_CHUNK
        src = x if d == "f" else x_rev
        seg = np.zeros((S, IN), np.float32)
        lo = j * CH - BURN
        if lo < 0:
            seg[-lo:] = src[0:(j + 1) * CH]
        else:
            seg[:] = src[lo:(j + 1) * CH]
        m = {
            "xT": np.ascontiguousarray(seg.T),
            "byh": byh,
        }
        m.update(per_dir[d])
        in_maps.append(m)
    return in_maps


def _run(in_maps, trace=False):
    nc = _get_program()
    return run_bass_kernel_spmd(nc, in_maps, list(range(N_CORES)), trace=trace)


def _assemble(results):
    y_f = np.concatenate(
        [results[j]["y"] for j in range(N_CHUNK)], axis=0
    )
    y_b_rev = np.concatenate(
        [results[N_CHUNK + j]["y"] for j in range(N_CHUNK)], axis=0
    )
    return (y_f + y_b_rev[::-1]).reshape(-1)


def kernel(**inputs) -> np.ndarray:
    in_maps = _make_in_maps(**inputs)
    res = _run(in_maps, trace=False)
    return _assemble(res.results)
